# revision 29
# baseline (speedup 1.0000x reference)
"""CrossAttention Trainium2 kernel.

Shapes (hardcoded from the problem spec):
  x  (32, 1024, 512) f32, xf (32, 77, 256) f32
  ln_g/ln_b (512,), tln_g/tln_b (256,)
  Wq (512,512), Wk (256,512), Wv (256,512), bq/bk/bv (512,)
  out y (32, 1024, 512) f32

Strategy:
  - Data-parallel over batch: 32 batches -> 8 cores x 4 batches. No collectives.
  - Host folds LayerNorm gamma/beta and biases into the projection weights
    (constant folding of parameters only), casts x/xf/weights to bf16.
  - Device per batch:
      xf: bn_stats layernorm -> transpose via PE -> K^T and [V|1] projections
      x:  bn_stats layernorm in natural layout -> xn to DRAM scratch ->
          DMA X-bar transpose load -> xn^T (D on partitions)
      Q:  q^T = Wq'^T @ xn^T  (bf16 matmuls, N=512 moving)
      S^T = k^T.T @ q^T per head (row-packed 2 heads per PE pass)
      P^T = exp(0.125 * S^T)  on ACT, bf16 out
      y   = P^T.T @ [V|1]  -> natural [T, 64+1] with softmax denominator in
            column 64; normalize during PSUM->SBUF copy; DMA out f32.
"""

import numpy as np
import ml_dtypes

import concourse.bass as bass
import concourse.bacc as bacc
import concourse.mybir as mybir
import concourse.tile as tile
from concourse.bass_utils import run_bass_kernel_spmd
from concourse.masks import make_identity

B, T, D, N, L, H = 32, 1024, 512, 77, 256, 8
HD = D // H           # 64
NCORES = 8
BPC = B // NCORES     # 4 batches per core
EPS = 1e-5
SCALE = 1.0 / np.sqrt(HD)  # 0.125

BF16 = mybir.dt.bfloat16
F32 = mybir.dt.float32

TC = T // 128         # 8 T-chunks per batch
DC = D // 128         # 4 D-chunks
LC = L // 128         # 2 L-chunks


class _Bacc(bacc.Bacc):
    """Bacc whose ACT-table chooser only finds Exp/Ln in the combined
    natural_log_exp_and_others set, so the kernel needs one table load
    instead of ping-ponging between exp_and_others and the ln set."""

    def insert_act_table_loads(self):
        import bass_rust as _br
        from concourse.hw_specs import get_activation_tables

        has_activation = any(
            isinstance(i, mybir.InstActivation)
            for blk in self.main_func.blocks
            for i in blk.instructions
        )
        if not has_activation:
            return
        pair = {
            mybir.ActivationFunctionType.Exp,
            mybir.ActivationFunctionType.Ln,
        }
        tables = []
        for name, fns in get_activation_tables(self.m.arch).items():
            if name != "natural_log_exp_and_others":
                fns = fns - pair
            tables.append((name, fns))
        _br.insert_act_table_loads(self, tables)


def _build(bpc=BPC, has_cq=False, has_ck=False, has_cv=False):
    nc = _Bacc("TRN2", target_bir_lowering=False, debug=False)

    xh = nc.dram_tensor("xh", (bpc, T, D), BF16, kind="ExternalInput")
    xfh = nc.dram_tensor("xfh", (bpc, N, L), BF16, kind="ExternalInput")
    wq = nc.dram_tensor("wq", (D, D), BF16, kind="ExternalInput")
    wk = nc.dram_tensor("wk", (L, D), BF16, kind="ExternalInput")
    wv = nc.dram_tensor("wv", (L, D), BF16, kind="ExternalInput")
    cq_d = nc.dram_tensor("cq", (1, D), F32, kind="ExternalInput") if has_cq else None
    ck_d = nc.dram_tensor("ck", (1, D), F32, kind="ExternalInput") if has_ck else None
    cv_d = nc.dram_tensor("cv", (1, D), BF16, kind="ExternalInput") if has_cv else None
    y = nc.dram_tensor("y", (bpc, T, D), F32, kind="ExternalOutput")

    with tile.TileContext(nc) as tc:
        _trace(tc, bpc, xh, xfh, wq, wk, wv, cq_d, ck_d, cv_d, y)
    nc.compile()
    return nc


def _trace(tc, bpc, xh, xfh, wq, wk, wv, cq_d, ck_d, cv_d, y):
    nc = tc.nc
    from contextlib import ExitStack

    ctx = ExitStack()
    with ctx:
        consts = ctx.enter_context(tc.tile_pool(name="consts", bufs=1))
        xpool = ctx.enter_context(tc.tile_pool(name="xpool", bufs=2))
        stats = ctx.enter_context(tc.tile_pool(name="stats", bufs=4))
        xnpool = ctx.enter_context(tc.tile_pool(name="xnpool", bufs=2))
        xfpool = ctx.enter_context(tc.tile_pool(name="xfpool", bufs=2))
        kvpool = ctx.enter_context(tc.tile_pool(name="kvpool", bufs=5))
        xntpool = ctx.enter_context(tc.tile_pool(name="xntpool", bufs=32))
        qpool = ctx.enter_context(tc.tile_pool(name="qpool", bufs=2))
        ptpool = ctx.enter_context(tc.tile_pool(name="ptpool", bufs=12))
        ypool = ctx.enter_context(tc.tile_pool(name="ypool", bufs=3))
        dramp = ctx.enter_context(tc.tile_pool(name="dramp", bufs=4, space="DRAM"))
        # PSUM pools: 8 banks total. fq 2 + st 3 + yps 3 = 8 (1-bank slots).
        fq = ctx.enter_context(tc.tile_pool(name="fq", bufs=2, space="PSUM"))
        stp = ctx.enter_context(tc.tile_pool(name="stp", bufs=3, space="PSUM"))
        yps = ctx.enter_context(tc.tile_pool(name="yps", bufs=3, space="PSUM"))

        # ---- constants ----
        wq_sb = consts.tile([128, DC, D], BF16, tag="wq")
        nc.gpsimd.dma_start(out=wq_sb, in_=wq.rearrange("(c p) d -> p c d", p=128))
        wk_sb = consts.tile([128, LC, D], BF16, tag="wk")
        nc.gpsimd.dma_start(out=wk_sb, in_=wk.rearrange("(c p) d -> p c d", p=128))
        wv_sb = consts.tile([128, LC, D], BF16, tag="wv")
        nc.gpsimd.dma_start(out=wv_sb, in_=wv.rearrange("(c p) d -> p c d", p=128))
        eps_t = consts.tile([128, 1], F32, tag="eps")
        nc.vector.memset(eps_t, EPS)
        ident = consts.tile([128, 128], BF16, tag="ident")
        make_identity(nc, ident)
        cq_sb = ck_sb = cv_sb = None
        if cq_d is not None:
            cq_sb = consts.tile([128, DC], F32, tag="cq")  # [dout_part, chunk]
            nc.gpsimd.dma_start(
                out=cq_sb, in_=cq_d.rearrange("o (c p) -> (o p) c", p=128)
            )
        if ck_d is not None:
            ck_sb = consts.tile([128, DC], F32, tag="ck")
            nc.gpsimd.dma_start(
                out=ck_sb, in_=ck_d.rearrange("o (c p) -> (o p) c", p=128)
            )
        if cv_d is not None:
            cv_sb = consts.tile([1, D], BF16, tag="cv")
            nc.gpsimd.dma_start(out=cv_sb, in_=cv_d)
            ones_row = consts.tile([1, N], BF16, tag="ones_row")
            nc.vector.memset(ones_row, 1.0)

        kT_b, vt_b, xnT_b = {}, {}, {}

        # ============ phase 1: xf layernorm + K^T + [V|1], per batch =====
        for b in range(bpc):
            xf_t = xfpool.tile([N, L], BF16, tag="xf")
            nc.gpsimd.dma_start(out=xf_t, in_=xfh[b])
            st6 = stats.tile([N, 6], F32, tag="fst6")
            nc.vector.bn_stats(out=st6, in_=xf_t)
            mv_f = stats.tile([N, 2], F32, tag="fmv")
            nc.vector.bn_aggr(out=mv_f, in_=st6)
            # rstd = exp(-0.5*ln(var+eps)): Ln/Exp share one ACT table set.
            rstd_f = stats.tile([N, 1], F32, tag="frstd")
            nc.scalar.activation(
                out=rstd_f, in_=mv_f[:, 1:2],
                func=mybir.ActivationFunctionType.Ln,
                bias=eps_t[:N], scale=1.0,
            )
            nc.scalar.activation(
                out=rstd_f, in_=rstd_f,
                func=mybir.ActivationFunctionType.Exp, scale=-0.5,
            )
            xfn = xfpool.tile([N, L], BF16, tag="xfn")
            nc.vector.tensor_scalar(
                out=xfn, in0=xf_t,
                scalar1=mv_f[:, 0:1], scalar2=rstd_f,
                op0=mybir.AluOpType.subtract, op1=mybir.AluOpType.mult,
            )
            xfnT = xfpool.tile([128, LC, N], BF16, tag="xfnT")
            for c in range(LC):
                tps = fq.tile([128, N], BF16, tag="fq")
                nc.tensor.transpose(
                    out=tps, in_=xfn[:, c * 128:(c + 1) * 128], identity=ident[:N, :N]
                )
                nc.vector.tensor_copy(out=xfnT[:, c, :], in_=tps)

            kT = kvpool.tile([128, DC, N], BF16, tag="kT")
            for dc in range(DC):
                kps = fq.tile([128, N], F32, tag="fq")
                for lc in range(LC):
                    nc.tensor.matmul(
                        kps,
                        lhsT=wk_sb[:, lc, dc * 128:(dc + 1) * 128],
                        rhs=xfnT[:, lc, :],
                        start=(lc == 0), stop=(lc == LC - 1),
                    )
                if ck_sb is not None:
                    nc.vector.tensor_scalar_add(
                        out=kps, in0=kps, scalar1=ck_sb[:, dc:dc + 1]
                    )
                nc.vector.tensor_copy(out=kT[:, dc, :], in_=kps)
            kT_b[b] = kT

            vps = fq.tile([N, D], F32, tag="fq")
            for lc in range(LC):
                nc.tensor.matmul(
                    vps, lhsT=xfnT[:, lc, :], rhs=wv_sb[:, lc, :],
                    start=(lc == 0), stop=(lc == LC - 1 and cv_sb is None),
                )
            if cv_sb is not None:
                nc.tensor.matmul(vps, lhsT=ones_row, rhs=cv_sb, start=False, stop=True)
            vt = kvpool.tile([N, H, HD + 1], BF16, tag="vt")
            nc.vector.tensor_copy(
                out=vt[:, :, 0:HD], in_=vps.rearrange("n (h d) -> n h d", h=H)
            )
            nc.vector.memset(vt[:, :, HD:HD + 1], 1.0)
            vt_b[b] = vt

        # ============ phase 2: x layernorm -> DRAM -> xbar transpose =====
        # Pipelined per T-half so batch 0's Q matmuls can start early.
        for b in range(bpc):
            xn_dram = dramp.tile([T, D], BF16, tag="xn")
            xnT = {}
            for hf in range(2):
                x_t = xpool.tile([128, 4, D], BF16, tag="x")
                nc.gpsimd.dma_start(
                    out=x_t,
                    in_=xh[b, hf * 512:(hf + 1) * 512].rearrange(
                        "(c p) d -> p c d", p=128
                    ),
                )
                mv4 = stats.tile([128, 4, 2], F32, tag="mv4")
                for c in range(4):
                    s6 = stats.tile([128, 6], F32, tag="xst6")
                    nc.vector.bn_stats(out=s6, in_=x_t[:, c, :])
                    nc.vector.bn_aggr(out=mv4[:, c, :], in_=s6)
                rstd4 = stats.tile([128, 4], F32, tag="rstd4")
                nc.scalar.activation(
                    out=rstd4, in_=mv4[:, :, 1:2],
                    func=mybir.ActivationFunctionType.Ln,
                    bias=eps_t, scale=1.0,
                )
                nc.scalar.activation(
                    out=rstd4, in_=rstd4,
                    func=mybir.ActivationFunctionType.Exp, scale=-0.5,
                )
                xn_h = xnpool.tile([128, 4, D], BF16, tag="xn")
                for c in range(4):
                    nc.vector.tensor_scalar(
                        out=xn_h[:, c, :], in0=x_t[:, c, :],
                        scalar1=mv4[:, c, 0:1], scalar2=rstd4[:, c:c + 1],
                        op0=mybir.AluOpType.subtract, op1=mybir.AluOpType.mult,
                    )
                nc.gpsimd.dma_start(
                    out=xn_dram[hf * 512:(hf + 1) * 512, :].rearrange(
                        "(c p) d -> p c d", p=128
                    ),
                    in_=xn_h,
                )
                for dc in range(DC):
                    xt = xntpool.tile([128, 512], BF16, tag="xnT")
                    nc.sync.dma_start(
                        out=xt,
                        in_=xn_dram[
                            hf * 512:(hf + 1) * 512, dc * 128:(dc + 1) * 128
                        ],
                        transpose=True,
                    )
                    xnT[(dc, hf)] = xt
            xnT_b[b] = xnT

        # ============ phase 3: Q proj, attention, PV, output =============
        for b in range(bpc):
            xnT, kT, vt = xnT_b[b], kT_b[b], vt_b[b]

            qnT = qpool.tile([128, DC, T], BF16, tag="qnT")
            for dc in range(DC):
                for hf in range(2):
                    qp = fq.tile([128, 512], F32, tag="fq")
                    for kc in range(DC):
                        nc.tensor.matmul(
                            qp,
                            lhsT=wq_sb[:, kc, dc * 128:(dc + 1) * 128],
                            rhs=xnT[(kc, hf)],
                            start=(kc == 0), stop=(kc == DC - 1),
                        )
                    if cq_sb is not None:
                        nc.vector.tensor_scalar_add(
                            out=qp, in0=qp, scalar1=cq_sb[:, dc:dc + 1]
                        )
                    nc.scalar.copy(
                        out=qnT[:, dc, hf * 512:(hf + 1) * 512], in_=qp
                    )

            # S^T / P^T per (head pair, T half); head h at kT chunk h//2,
            # partition offset 64*(h%2); row-packed via tile_position.
            pt_tiles = {}
            for hp in range(H // 2):
                for hf in range(2):
                    pt = ptpool.tile([N, 2, 512], BF16, tag="pt")
                    for sub in range(2):
                        po = 64 * sub
                        stt = stp.tile([N, 512], F32, tag="st")
                        nc.tensor.matmul(
                            stt,
                            lhsT=kT[po:po + 64, hp, :],
                            rhs=qnT[po:po + 64, hp, hf * 512:(hf + 1) * 512],
                            start=True, stop=True,
                            tile_position=(po, 0),
                        )
                        nc.scalar.activation(
                            out=pt[:, sub, :], in_=stt,
                            func=mybir.ActivationFunctionType.Exp,
                            scale=float(SCALE),
                        )
                    pt_tiles[(hp, hf)] = pt

            for c in range(TC):
                hf, ci = c // 4, c % 4
                yp0 = yps.tile([128, 4, HD + 1], F32, tag="yp")
                yp1 = yps.tile([128, 4, HD + 1], F32, tag="yp")
                ypl = (yp0, yp1)
                for h in range(H):
                    pt = pt_tiles[(h // 2, hf)]
                    nc.tensor.matmul(
                        ypl[h // 4][:, h % 4, :],
                        lhsT=pt[:, h % 2, ci * 128:(ci + 1) * 128],
                        rhs=vt[:, h, :],
                        start=True, stop=True,
                    )
                y_sb = ypool.tile([128, D], F32, tag="y")
                for j in range(2):
                    rs = stats.tile([128, 4], F32, tag="rs")
                    nc.vector.reciprocal(out=rs, in_=ypl[j][:, :, HD:HD + 1])
                    rs_ap = rs[:, :]
                    rs_b = bass.AP(
                        tensor=rs_ap.tensor, offset=rs_ap.offset,
                        ap=[rs_ap.ap[0], rs_ap.ap[1], [0, HD]],
                    )
                    nc.vector.tensor_mul(
                        out=y_sb.rearrange("p (j h d) -> p j h d", j=2, h=4)[:, j],
                        in0=ypl[j][:, :, 0:HD],
                        in1=rs_b,
                    )
                nc.gpsimd.dma_start(
                    out=y[b, c * 128:(c + 1) * 128, :], in_=y_sb
                )


_CACHE = {}
TRACE = False          # set True to capture an NTFF profile on core 0
LAST_RESULTS = None    # BassKernelResults of the most recent kernel() call


def _get_nc(key):
    if key not in _CACHE:
        _CACHE[key] = _build(*key)
    return _CACHE[key]


def kernel(x, xf, ln_g, ln_b, tln_g, tln_b, Wq, bq, Wk, bk, Wv, bv):
    x = np.asarray(x, np.float32)
    xf = np.asarray(xf, np.float32)
    # Fold layernorm affine + biases into the projections (f32 host math).
    wq_f = np.asarray(ln_g, np.float32)[:, None] * np.asarray(Wq, np.float32)
    cq = np.asarray(ln_b, np.float32) @ np.asarray(Wq, np.float32) + np.asarray(bq, np.float32)
    wk_f = np.asarray(tln_g, np.float32)[:, None] * np.asarray(Wk, np.float32)
    ck = np.asarray(tln_b, np.float32) @ np.asarray(Wk, np.float32) + np.asarray(bk, np.float32)
    wv_f = np.asarray(tln_g, np.float32)[:, None] * np.asarray(Wv, np.float32)
    cv = np.asarray(tln_b, np.float32) @ np.asarray(Wv, np.float32) + np.asarray(bv, np.float32)

    has_cq = bool(np.any(cq != 0))
    has_ck = bool(np.any(ck != 0))
    has_cv = bool(np.any(cv != 0))
    nc = _get_nc((BPC, has_cq, has_ck, has_cv))

    bf = ml_dtypes.bfloat16
    wq_b = wq_f.astype(bf)
    wk_b = wk_f.astype(bf)
    wv_b = wv_f.astype(bf)
    x_b = x.astype(bf)
    xf_b = xf.astype(bf)

    in_maps = []
    for i in range(NCORES):
        m = {
            "xh": np.ascontiguousarray(x_b[i * BPC:(i + 1) * BPC]),
            "xfh": np.ascontiguousarray(xf_b[i * BPC:(i + 1) * BPC]),
            "wq": wq_b, "wk": wk_b, "wv": wv_b,
        }
        if has_cq:
            m["cq"] = cq.reshape(1, D)
        if has_ck:
            m["ck"] = ck.reshape(1, D)
        if has_cv:
            m["cv"] = cv.reshape(1, D).astype(bf)
        in_maps.append(m)

    global LAST_RESULTS
    res = run_bass_kernel_spmd(
        nc, in_maps, core_ids=list(range(NCORES)), trace=TRACE
    )
    LAST_RESULTS = res
    out = np.concatenate([r["y"] for r in res.results], axis=0)
    return out.astype(np.float32)
